# revision 11
# baseline (speedup 1.0000x reference)
"""FASTMultiHeadAttention (polynomial softmax + RPE bias, causal) on 8 trn2 cores.

Math per (b,h):   s[i,j] = q_i.k_j + q_i.rpe[n-1+i-j]
                  score  = 1 + s + 0.5 s^2    (= 0.5[(s+1)^2 + 1], 0.5 cancels)
                  o_i    = sum_{j<=i} score v_j / sum_{j<=i} score

Device pipeline per (b,h)  [B*H = 32 units, 4 per core]:
  - m2r band  = qT-upper @ rpeR        (PE rows 64-127, K=64 row-packed)
  - bias      = batched shear-read of m2r band (4x 3-dim SBUF->SBUF DMA)
  - psum_s    = qT-lower @ kT          (PE rows 0-63)
  - wrow      = (psum_s + 1) + bias    (DVE scalar_tensor_tensor -> bf16)
  - diag mask via gpsimd.affine_select
  - wT blocks = PE transpose (batched per row-tile pair), Square evac (ACT)
  - oT[65,256] += va_J.T @ scoreT      (PE, accumulated over J; col 64 = ones)
  - oaug      = oT + ptT               (DVE, host-precomputed prefix of vaug)
  - po        = transpose-back batches; o = po * recip(denom) in one
                broadcast tensor_tensor per 4-tile group (DVE)
"""

import sys

if "/opt/trn_rl_repo" not in sys.path:
    sys.path.insert(0, "/opt/trn_rl_repo")

import ml_dtypes
import numpy as np

import bass_rust
import concourse.bacc as bacc
import concourse.bass as bass
import concourse.mybir as mybir
import concourse.tile as tile
from concourse.bass_utils import run_bass_kernel_spmd

F32 = mybir.dt.float32
BF16 = mybir.dt.bfloat16

B, H, N, D = 2, 16, 1024, 64
NBH = B * H  # 32
N_CORES = 8
BH_PER_CORE = NBH // N_CORES  # 4
NT = N // 128  # 8 row tiles
MROW = 1152  # m2r band row stride (>= max band width 1151)
RPE_W = 1152  # width of reversed rpe band (1151 + pad col)

# bias_all column offsets per tile (pair-uniform widths 256*(t+1))
BOFF = [0, 256, 512, 1024, 1536, 2304, 3072, 4096]
BIAS_W = 5120


def _chunks(total):
    out = []
    c = 0
    while c < total:
        out.append((c, min(512, total - c)))
        c += 512
    return out


S_CHUNKS = {I: _chunks(128 * (I + 1)) for I in range(8)}
M2R_CHUNKS = {I: _chunks(255 + 128 * I) for I in range(8)}


def _ap(t_ap, pairs, offset=0):
    cp = t_ap.copy()
    cp.ap = bass_rust.VecI64Pair(pairs)
    cp.offset = offset
    return cp


def build_program():
    nc = bacc.Bacc(
        "TRN2", target_bir_lowering=False, debug=False, num_devices=N_CORES
    )

    qT_d = nc.dram_tensor("qT", [BH_PER_CORE, 128, N], BF16, kind="ExternalInput").ap()
    kT_d = nc.dram_tensor("kT", [BH_PER_CORE, 64, N], BF16, kind="ExternalInput").ap()
    va_d = nc.dram_tensor("va", [BH_PER_CORE, 128, NT * 65], BF16, kind="ExternalInput").ap()
    pt_d = nc.dram_tensor("pt", [BH_PER_CORE, 65, N], F32, kind="ExternalInput").ap()
    rpe_d = nc.dram_tensor("rpeR", [64, RPE_W], BF16, kind="ExternalInput").ap()
    idn_d = nc.dram_tensor("idn", [128, 128], BF16, kind="ExternalInput").ap()
    o_d = nc.dram_tensor("o", [BH_PER_CORE, 65, N], F32, kind="ExternalOutput").ap()

    with tile.TileContext(nc) as tc:
        with (
            tc.tile_pool(name="const", bufs=1) as cpool,
            tc.tile_pool(name="io", bufs=2) as io,
            tc.tile_pool(name="m2r", bufs=2) as m2rp,
            tc.tile_pool(name="biasp", bufs=2) as bp,
            tc.tile_pool(name="wrow", bufs=2) as wp,
            tc.tile_pool(name="sct", bufs=2) as scp,
            tc.tile_pool(name="fin", bufs=2) as fp,
            tc.tile_pool(name="psmm", bufs=4, space="PSUM") as ps_mm,
            tc.tile_pool(name="pswt", bufs=2, space="PSUM") as ps_wt,
            tc.tile_pool(name="psot", bufs=2, space="PSUM") as ps_ot,
        ):
            rpeR = cpool.tile([128, RPE_W], BF16)
            nc.gpsimd.dma_start(rpeR[0:64, :], rpe_d[:])
            nc.gpsimd.dma_start(rpeR[64:128, :], rpe_d[:])

            idn = cpool.tile([128, 128], BF16)
            nc.gpsimd.dma_start(idn[:], idn_d[:])

            def load_unit(m):
                qT = io.tile([128, N], BF16, tag="qT")
                nc.gpsimd.dma_start(qT[:], qT_d[m])
                kT = io.tile([64, N], BF16, tag="kT")
                nc.gpsimd.dma_start(kT[:], kT_d[m])
                va = io.tile([128, NT * 65], BF16, tag="va")
                nc.gpsimd.dma_start(va[:], va_d[m])
                pt = io.tile([65, N], F32, tag="pt")
                nc.sync.dma_start(pt[:], pt_d[m])
                return {"qT": qT, "kT": kT, "va": va, "pt": pt}

            def emit_m2r(u):
                """Phase A: m2r band matmuls + evac + batched shears.

                One m2r buffer per tile PAIR so the shear's flat-AP read
                footprint never WAR-blocks later pairs' evacuations."""
                qT = u["qT"]
                bias_all = bp.tile([128, BIAS_W], BF16, tag="bias")
                u["bias_all"] = bias_all
                flip = 0
                for t in range(NT // 2):
                    m2r_pair = m2rp.tile([128, 2 * MROW], BF16, tag=f"m2r{t}")
                    for half in range(2):
                        I = 2 * t + half
                        u0 = 896 - 128 * I
                        for c, wd in M2R_CHUNKS[I]:
                            pm = ps_mm.tile([128, 512], F32, tag="mm")
                            nc.tensor.matmul(
                                pm[:, :wd],
                                qT[64:128, 128 * I : 128 * (I + 1)],
                                rpeR[64:128, u0 + c : u0 + c + wd],
                                start=True,
                                stop=True,
                                tile_position=(64, 0),
                            )
                            dst = m2r_pair[:, MROW * half + c : MROW * half + c + wd]
                            # alternate evac engine for ACT/DVE balance
                            if flip % 2 == 0:
                                nc.scalar.copy(dst, pm[:, :wd])
                            else:
                                nc.vector.tensor_copy(dst, pm[:, :wd])
                            flip += 1
                    wt = 256 * (t + 1)
                    nc.sync.dma_start(
                        _ap(
                            bias_all[:],
                            [[BIAS_W, 128],
                             [BOFF[2 * t + 1] - BOFF[2 * t], 2], [1, wt]],
                            offset=BOFF[2 * t],
                        ),
                        _ap(
                            m2r_pair[:],
                            [[2 * MROW - 1, 128], [MROW, 2], [1, wt]],
                            offset=127,
                        ),
                    )

            def emit_s(u, I):
                qT, kT, bias_all = u["qT"], u["kT"], u["bias_all"]
                wrow = wp.tile([128, N], BF16, tag=f"w{I}")
                u[f"wrow{I}"] = wrow
                for c, wd in S_CHUNKS[I]:
                    psz = ps_mm.tile([128, 512], F32, tag="mm")
                    nc.tensor.matmul(
                        psz[:, :wd],
                        qT[0:64, 128 * I : 128 * (I + 1)],
                        kT[:, c : c + wd],
                        start=True,
                        stop=True,
                        tile_position=(0, 0),
                    )
                    nc.vector.scalar_tensor_tensor(
                        wrow[:, c : c + wd],
                        psz[:, :wd],
                        1.0,
                        bias_all[:, BOFF[I] + c : BOFF[I] + c + wd],
                        mybir.AluOpType.add,
                        mybir.AluOpType.add,
                    )
                nc.gpsimd.affine_select(
                    wrow[:, 128 * I : 128 * (I + 1)],
                    wrow[:, 128 * I : 128 * (I + 1)],
                    pattern=[[-1, 128]],
                    compare_op=mybir.AluOpType.is_ge,
                    fill=0.0,
                    base=0,
                    channel_multiplier=1,
                )

            def emit_pair(u, p):
                va, pt = u["va"], u["pt"]
                scoreT = scp.tile([128, 2048], BF16, tag="scoreT")
                for half in range(2):
                    I = 2 * p + half
                    wrow = u.pop(f"wrow{I}")
                    for c, wd in S_CHUNKS[I]:
                        pw = ps_wt.tile([128, 512], BF16, tag="wt")
                        for bofs in range(0, wd, 128):
                            nc.tensor.transpose(
                                pw[:, bofs : bofs + 128],
                                wrow[:, c + bofs : c + bofs + 128],
                                idn[:],
                            )
                        nc.scalar.activation(
                            scoreT[:, 1024 * half + c : 1024 * half + c + wd],
                            pw[:, :wd],
                            mybir.ActivationFunctionType.Square,
                        )
                nc.vector.memset(
                    scoreT[:, 128 * (2 * p + 1) : 128 * (2 * p + 1) + 128].bitcast(F32),
                    0.0,
                )
                pot = ps_ot.tile([65, 256], F32, tag="ot")
                njs = 2 * p + 2
                for J in range(njs):
                    rhs = _ap(
                        scoreT[:], [[2048, 128], [1024, 2], [1, 128]], offset=128 * J
                    )
                    out3 = _ap(pot[:], [[256, 65], [128, 2], [1, 128]])
                    nc.tensor.matmul(
                        out3,
                        va[:, 65 * J : 65 * (J + 1)],
                        rhs,
                        start=(J == 0),
                        stop=(J == njs - 1),
                    )
                if "oaug" not in u:
                    oaug_t = fp.tile([65, N], F32, tag="oaug")
                    u["oaug"] = oaug_t
                nc.vector.tensor_add(
                    u["oaug"][:, 256 * p : 256 * (p + 1)],
                    pot[:],
                    pt[:, 256 * p : 256 * (p + 1)],
                )

            def emit_out(u, m):
                nc.sync.dma_start(o_d[m], u["oaug"][:])

            # software pipeline: phase A of unit m+1 overlaps the tail of unit m
            units = {}
            units[0] = load_unit(0)
            emit_m2r(units[0])
            for m in range(BH_PER_CORE):
                u = units[m]
                last = m + 1 == BH_PER_CORE
                if not last:
                    emit_s(u, 0)
                    emit_s(u, 1)
                    emit_s(u, 2)
                    emit_s(u, 3)
                    emit_pair(u, 0)
                    emit_s(u, 4)
                    emit_s(u, 5)
                    emit_pair(u, 1)
                    emit_s(u, 6)
                    emit_s(u, 7)
                    emit_pair(u, 2)
                    units[m + 1] = load_unit(m + 1)
                    emit_m2r(units[m + 1])
                    emit_pair(u, 3)
                else:
                    # drain order: biggest pairs first so the tail chain is short
                    emit_s(u, 6)
                    emit_s(u, 7)
                    emit_s(u, 4)
                    emit_s(u, 5)
                    emit_pair(u, 3)
                    emit_s(u, 2)
                    emit_s(u, 3)
                    emit_pair(u, 2)
                    emit_s(u, 0)
                    emit_s(u, 1)
                    emit_pair(u, 1)
                    emit_pair(u, 0)
                emit_out(u, m)
                del units[m]

    nc.compile()
    return nc


_NC_CACHE = {}


def get_program():
    if "nc" not in _NC_CACHE:
        _NC_CACHE["nc"] = build_program()
    return _NC_CACHE["nc"]


def prepare_inputs(q, k, v, rpe_matrix):
    """Host-side prep: returns per-core input maps (device-layout tensors)."""
    q = np.asarray(q, dtype=np.float32).reshape(NBH, N, D)
    k = np.asarray(k, dtype=np.float32).reshape(NBH, N, D)
    v = np.asarray(v, dtype=np.float32).reshape(NBH, N, D)
    rpe = np.asarray(rpe_matrix, dtype=np.float32)

    def rbf(x):
        return x.astype(ml_dtypes.bfloat16)

    qT1 = rbf(np.ascontiguousarray(q.transpose(0, 2, 1)))  # [32, 64, 1024]
    qT = np.concatenate([qT1, qT1], axis=1)  # [32, 128, 1024] both PE halves
    kT = rbf(np.ascontiguousarray(k.transpose(0, 2, 1)))
    va0 = np.concatenate([v, np.ones((NBH, N, 1), np.float32)], axis=2).astype(
        ml_dtypes.bfloat16
    ).astype(np.float32)  # [32,1024,65] (bf16-rounded)
    # device layout: va[p, 65a+d] = vaug[128a+p, d]
    va = np.ascontiguousarray(
        va0.reshape(NBH, NT, 128, 65).transpose(0, 2, 1, 3)
    ).reshape(NBH, 128, NT * 65)
    pt = np.ascontiguousarray(
        np.cumsum(va0.astype(np.float64), axis=1).transpose(0, 2, 1)
    ).astype(np.float32)  # [32, 65, 1024]
    va = va.astype(ml_dtypes.bfloat16)

    # reversed rpe band: rpeR[:, u] = rpe[2046 - u] for u in [0, 1151)
    rpeR = np.zeros((64, RPE_W), np.float32)
    rpeR[:, :1151] = rpe[2046:895:-1].T
    rpeR = rpeR.astype(ml_dtypes.bfloat16)
    idn = np.eye(128, dtype=np.float32).astype(ml_dtypes.bfloat16)

    in_maps = []
    for c in range(N_CORES):
        sl = slice(c * BH_PER_CORE, (c + 1) * BH_PER_CORE)
        in_maps.append(
            {
                "qT": np.ascontiguousarray(qT[sl]),
                "kT": np.ascontiguousarray(kT[sl]),
                "va": np.ascontiguousarray(va[sl]),
                "pt": np.ascontiguousarray(pt[sl]),
                "rpeR": rpeR,
                "idn": idn,
            }
        )
    return in_maps


def run(q, k, v, rpe_matrix, trace=False):
    nc = get_program()
    in_maps = prepare_inputs(q, k, v, rpe_matrix)
    res = run_bass_kernel_spmd(nc, in_maps, list(range(N_CORES)), trace=trace)
    # device returns oaugT [65, N] per unit: rows 0-63 = numerator^T,
    # row 64 = denominator; final division on host (exact fp32).
    outs = []
    for c in range(N_CORES):
        od = res.results[c]["o"]  # [4, 65, 1024] f32
        num = od[:, :64, :]      # [4, 64, N]
        den = od[:, 64:65, :]    # [4, 1, N]
        outs.append(np.ascontiguousarray((num / den).transpose(0, 2, 1)))
    o = np.concatenate(outs, axis=0).reshape(B, H, N, D)
    return o, res


def kernel(q, k, v, drop_noise=None, rpe_matrix=None, p=2, **kw):
    o, _ = run(q, k, v, rpe_matrix)
    return o


if __name__ == "__main__":
    rng = np.random.default_rng(0)
    q = rng.standard_normal((B, H, N, D), dtype=np.float32)
    k = rng.standard_normal((B, H, N, D), dtype=np.float32)
    v = rng.standard_normal((B, H, N, D), dtype=np.float32)
    rpe = rng.standard_normal((2 * N - 1, D), dtype=np.float32)
    o, _ = run(q, k, v, rpe)
    print("out", o.shape, o.dtype, np.abs(o).max())
